# revision 29
# baseline (speedup 1.0000x reference)
"""Trainium2 Bass kernel for AttentionMM.

Reference computation (per batch b, T=E=512):
    alpha = softmax(x1 @ x2^T, axis=-1)              # [T, T]
    a1t   = alpha^T @ x2                             # [T, E]
    a2t   = alpha @ x1                               # [T, E]
    o1    = mean_t tanh(x1 @ U1 + a1t @ V1)          # [E]
    o2    = mean_t tanh(x2 @ U2 + a2t @ V2)          # [E]
    out[b] = concat(o1, o2)                          # [2E]

Sharding: data-parallel over batch across 8 NeuronCores (4 batches/core),
U1/U2/V1/V2 replicated.  No collectives needed; shard/gather on host.

All matmuls run in bf16 (full TensorEngine rate); accumulation is f32 in
PSUM.  Per-batch dataflow keeps everything in "transposed" layouts so that
every contraction lands on the partition axis and the final mean over T is
a free-axis reduction (done for free by activation(accum_out=...)):
    S      = x1 @ x2^T          via lhsT=x1^T blocks, rhs=x2^T   -> [t, s]
    alpha  = softmax rows (reduce_max(negate) -> Exp(bias=-max, accum_out)
             -> reciprocal -> tensor_scalar_mul)
    alphaT = PE-transpose of alpha blocks (16x 128x128)
    a1^T   = lhsT=x2 blocks,  rhs=alpha                          -> [e, t]
    a2^T   = lhsT=x1 blocks,  rhs=alphaT                         -> [e, t]
    o1pre^T= lhsT=U1 blocks, rhs=x1^T  (+) lhsT=V1 blocks, rhs=a1^T
    o2pre^T= lhsT=U2 blocks, rhs=x2^T  (+) lhsT=V2 blocks, rhs=a2^T
    tanh with accum_out -> per-partition sums -> staged, one final PE
    transpose + scale by 1/T -> single contiguous DMA out.
"""

import sys

if "/opt/trn_rl_repo" not in sys.path:
    sys.path.insert(0, "/opt/trn_rl_repo")

import ml_dtypes
import numpy as np

B, T, E = 32, 512, 512
NCORES = 8
BL = B // NCORES  # batches per core
P = 128
NT = T // P
NE = E // P

_CACHE = {}


def _build():
    from contextlib import ExitStack

    import concourse.bass as bass
    import concourse.tile as tile
    from concourse import bacc, mybir
    from concourse.masks import make_identity

    bf16 = mybir.dt.bfloat16
    f32 = mybir.dt.float32
    AF = mybir.ActivationFunctionType
    AX = mybir.AxisListType

    nc = bacc.Bacc(
        "TRN2",
        target_bir_lowering=False,
        debug=False,
        enable_asserts=False,
        num_devices=NCORES,
    )

    x1_d = nc.dram_tensor("x1", [BL, T, E], bf16, kind="ExternalInput")
    x2_d = nc.dram_tensor("x2", [BL, T, E], bf16, kind="ExternalInput")
    # host-pretransposed copies: x1t[b, e, t] = x1[b, t, e]
    x1t_d = nc.dram_tensor("x1t", [BL, E, T], bf16, kind="ExternalInput")
    x2t_d = nc.dram_tensor("x2t", [BL, E, T], bf16, kind="ExternalInput")
    w_d = {
        nm: nc.dram_tensor(nm, [E, E], bf16, kind="ExternalInput")
        for nm in ("u1", "v1", "u2", "v2")
    }
    out_d = nc.dram_tensor("out", [BL, 2 * E], f32, kind="ExternalOutput")

    with tile.TileContext(nc) as tc, ExitStack() as ctx:
        const = ctx.enter_context(tc.tile_pool(name="const", bufs=1))
        wpool = ctx.enter_context(tc.tile_pool(name="wts", bufs=1))
        xpool = ctx.enter_context(tc.tile_pool(name="x", bufs=BL))
        apool = ctx.enter_context(tc.tile_pool(name="alpha", bufs=2))
        cpool = ctx.enter_context(tc.tile_pool(name="attn", bufs=2))
        spool = ctx.enter_context(tc.tile_pool(name="stats", bufs=16))
        tpool = ctx.enter_context(tc.tile_pool(name="trash", bufs=2))
        stgp = ctx.enter_context(tc.tile_pool(name="stage", bufs=1))
        ps_s = ctx.enter_context(tc.tile_pool(name="ps_s", bufs=2, space="PSUM"))
        ps_t = ctx.enter_context(tc.tile_pool(name="ps_t", bufs=2, space="PSUM"))
        ps_a = ctx.enter_context(tc.tile_pool(name="ps_a", bufs=2, space="PSUM"))
        ps_o = ctx.enter_context(tc.tile_pool(name="ps_o", bufs=2, space="PSUM"))

        id_bf = const.tile([P, P], bf16, tag="id_bf")
        make_identity(nc, id_bf[:])
        id_f32 = const.tile([P, P], f32, tag="id_f32")
        make_identity(nc, id_f32[:])

        # col = b*8 + half*4 + f  ->  out[b, half*512 + f*128 : +128]
        stage = stgp.tile([P, 8 * BL], f32, tag="stage")

        def load_xt(b):
            """Transposed-layout loads (host pre-transposed): plain DMAs on
            the sync ring.  x1t tile[p, e, t] = x1[b, t, e*128+p]."""
            x1t = xpool.tile([P, NE, T], bf16, tag="x1t")
            x2t = xpool.tile([P, NE, T], bf16, tag="x2t")
            nc.sync.dma_start(x2t[:], x2t_d.ap()[b].rearrange("(a p) t -> p a t", p=P))
            nc.sync.dma_start(x1t[:], x1t_d.ap()[b].rearrange("(a p) t -> p a t", p=P))
            return x1t, x2t

        def load_xn(b):
            """Natural-layout loads on the gpsimd SWDGE queue (parallel with
            the sync ring; safe now that no xbar transposes are in flight)."""
            x1n = xpool.tile([P, NT, E], bf16, tag="x1n")
            x2n = xpool.tile([P, NT, E], bf16, tag="x2n")
            # x2n first: it is mm3's lhsT, the first consumer after softmax
            nc.gpsimd.dma_start(x2n[:], x2_d.ap()[b].rearrange("(i p) e -> p i e", p=P))
            nc.gpsimd.dma_start(x1n[:], x1_d.ap()[b].rearrange("(i p) e -> p i e", p=P))
            return x1n, x2n

        def load_w():
            ws = {}
            for nm, d in w_d.items():
                w = wpool.tile([P, NE, E], bf16, tag=nm)
                nc.scalar.dma_start(w[:], d.ap().rearrange("(a p) f -> p a f", p=P))
                ws[nm] = w
            return ws

        def s_phase(X):
            """S = x1 @ x2^T, then row softmax -> alpha [t-part, s-free] bf16."""
            _, _, x1t, x2t = X
            alpha = apool.tile([P, NT, T], bf16, tag="alpha")
            for i in range(NT):
                ps = ps_s.tile([P, T], f32, tag="s")
                for e in range(NE):
                    nc.tensor.matmul(
                        ps[:],
                        lhsT=x1t[:, e, i * P : (i + 1) * P],
                        rhs=x2t[:, e, :],
                        start=(e == 0),
                        stop=(e == NE - 1),
                    )
                mneg = spool.tile([P, 1], f32, tag="mneg")
                nc.vector.reduce_max(out=mneg[:], in_=ps[:], axis=AX.X, negate=True)
                ssum = spool.tile([P, 1], f32, tag="ssum")
                nc.scalar.activation(
                    alpha[:, i, :], ps[:], AF.Exp, bias=mneg[:], accum_out=ssum[:]
                )
                rcol = spool.tile([P, 1], f32, tag="rcol")
                nc.vector.reciprocal(rcol[:], ssum[:])
                nc.vector.tensor_scalar_mul(alpha[:, i, :], alpha[:, i, :], rcol[:])
            return alpha

        def transpose_alpha(alpha):
            """alphaT[j-part, t-free] via 16 PE block transposes.  Emitted
            ahead of the NEXT batch's S matmuls so the PE hits it while the
            softmax epilogue is long done."""
            alphaT = apool.tile([P, NT, T], bf16, tag="alphaT")
            for j in range(NT):
                pst = ps_t.tile([P, T], bf16, tag="t")
                for i in range(NT):
                    nc.tensor.transpose(
                        pst[:, i * P : (i + 1) * P],
                        alpha[:, i, j * P : (j + 1) * P],
                        id_bf[:],
                    )
                nc.vector.tensor_copy(out=alphaT[:, j, :], in_=pst[:])
            return alphaT

        def rest_phase(b, X, alpha, alphaT, ws):
            x1n, x2n, x1t, x2t = X
            # a1^T[e, t] = sum_k x2[k, e] * alpha[k, t]
            a1 = cpool.tile([P, NE, T], bf16, tag="a1")
            for e in range(NE):
                pa = ps_a.tile([P, T], f32, tag="a")
                for i in range(NT):
                    nc.tensor.matmul(
                        pa[:],
                        lhsT=x2n[:, i, e * P : (e + 1) * P],
                        rhs=alpha[:, i, :],
                        start=(i == 0),
                        stop=(i == NT - 1),
                    )
                nc.vector.tensor_copy(out=a1[:, e, :], in_=pa[:])
            # a2^T[e, t] = sum_s x1[s, e] * alphaT[s, t]
            a2 = cpool.tile([P, NE, T], bf16, tag="a2")
            for e in range(NE):
                pa = ps_a.tile([P, T], f32, tag="a")
                for j in range(NT):
                    nc.tensor.matmul(
                        pa[:],
                        lhsT=x1n[:, j, e * P : (e + 1) * P],
                        rhs=alphaT[:, j, :],
                        start=(j == 0),
                        stop=(j == NT - 1),
                    )
                nc.vector.tensor_copy(out=a2[:, e, :], in_=pa[:])
            # o{1,2}pre^T[f, t] = sum_e U[e,f] x^T[e,t] + sum_e V[e,f] a^T[e,t]
            for half, (wu, wv, xt, at) in enumerate(
                (("u1", "v1", x1t, a1), ("u2", "v2", x2t, a2))
            ):
                for f in range(NE):
                    po = ps_o.tile([P, T], f32, tag="o")
                    for e in range(NE):
                        nc.tensor.matmul(
                            po[:],
                            lhsT=ws[wu][:, e, f * P : (f + 1) * P],
                            rhs=xt[:, e, :],
                            start=(e == 0),
                            stop=False,
                        )
                    for e in range(NE):
                        nc.tensor.matmul(
                            po[:],
                            lhsT=ws[wv][:, e, f * P : (f + 1) * P],
                            rhs=at[:, e, :],
                            start=False,
                            stop=(e == NE - 1),
                        )
                    trash = tpool.tile([P, T], bf16, tag="trash")
                    col = b * 8 + half * NE + f
                    nc.scalar.activation(
                        trash[:],
                        po[:],
                        AF.Tanh,
                        accum_out=stage[:, col : col + 1],
                    )

        # Loads spread across the three DMA queues, in need-time order:
        # transposed layouts (earliest consumers) back-to-back on sync,
        # naturals b0-b1 + weights next (gpsimd/scalar), naturals b2-b3 last.
        Xt = {}
        Xn = {}
        for b in range(BL):
            Xt[b] = load_xt(b)
        Xn[0] = load_xn(0)
        Xn[1] = load_xn(1)
        ws = load_w()
        Xn[2] = load_xn(2)
        Xn[3] = load_xn(3)

        # Warm the PE (HAM clock gate) while batch-0 data is in flight:
        # a few throwaway matmuls on the identity, then on x2t as it lands.
        warm_ps = ps_t.tile([P, T], f32, tag="t")
        for k in range(6):
            nc.tensor.matmul(
                warm_ps[:, :P], lhsT=id_bf[:], rhs=id_bf[:], start=True, stop=True
            )
        for k in range(8):
            nc.tensor.matmul(
                warm_ps[:], lhsT=id_bf[:], rhs=Xt[0][1][:, k % NE, :],
                start=True, stop=True,
            )

        Xs = [Xn[b] + Xt[b] for b in range(BL)]  # (x1n, x2n, x1t, x2t)

        # Software pipeline; PE stream per step b:
        #   transpose_alpha(b-1) | S(b) | rest(b-1)
        prev_alpha = None
        for b in range(BL):
            if prev_alpha is not None:
                prev_alphaT = transpose_alpha(prev_alpha)
            alpha = s_phase(Xs[b])
            if prev_alpha is not None:
                rest_phase(b - 1, Xs[b - 1], prev_alpha, prev_alphaT, ws)
            prev_alpha = alpha
        prev_alphaT = transpose_alpha(prev_alpha)
        rest_phase(BL - 1, Xs[BL - 1], prev_alpha, prev_alphaT, ws)

        # Final: transpose stage [128, 8*BL] -> [8*BL, 128], scale by 1/T, DMA out
        pfin = ps_s.tile([8 * BL, P], f32, tag="s")
        nc.tensor.transpose(pfin[:], stage[:], id_f32[:])
        fin = tpool.tile([8 * BL, P], f32, tag="fin")
        nc.scalar.mul(fin[:], pfin[:], 1.0 / T)
        nc.sync.dma_start(out_d.ap().rearrange("b (x f) -> (b x) f", f=P), fin[:])

    nc.compile()
    return nc


def _get_nc():
    if "nc" not in _CACHE:
        _CACHE["nc"] = _build()
    return _CACHE["nc"]


def _make_in_maps(inputs):
    bf = ml_dtypes.bfloat16
    x1 = np.asarray(inputs["x1"], dtype=np.float32).astype(bf)
    x2 = np.asarray(inputs["x2"], dtype=np.float32).astype(bf)
    wmap = {
        nm: np.ascontiguousarray(np.asarray(inputs[NM], dtype=np.float32)).astype(bf)
        for nm, NM in (("u1", "U1"), ("v1", "V1"), ("u2", "U2"), ("v2", "V2"))
    }
    in_maps = []
    for c in range(NCORES):
        sl = slice(c * BL, (c + 1) * BL)
        m = {
            "x1": np.ascontiguousarray(x1[sl]),
            "x2": np.ascontiguousarray(x2[sl]),
            "x1t": np.ascontiguousarray(x1[sl].transpose(0, 2, 1)),
            "x2t": np.ascontiguousarray(x2[sl].transpose(0, 2, 1)),
        }
        m.update(wmap)
        in_maps.append(m)
    return in_maps


def _run(inputs, trace=False, **kw):
    from concourse.bass_utils import run_bass_kernel_spmd

    nc = _get_nc()
    res = run_bass_kernel_spmd(
        nc, _make_in_maps(inputs), core_ids=list(range(NCORES)), trace=trace, **kw
    )
    out = np.concatenate([r["out"] for r in res.results], axis=0)
    return np.asarray(out, dtype=np.float32), res


def kernel(**inputs):
    out, _ = _run(inputs, trace=False)
    return out
